# revision 29
# baseline (speedup 1.0000x reference)
"""GCN (2x GCNConv + global_mean_pool + FC + sigmoid) on 8 TRN2 NeuronCores.

Sharding: nodes (and incident edges, by dst) are partitioned across 8 cores.
Each core computes the feature transform + message aggregation for its 6250
dst nodes; hs (dinv-scaled transformed features, fp8-e4m3) is AllGathered
between layers; the [16, 512] FC partials are AllReduced at the end.

fp8 message path: hs rows are stored pair-packed ([n/2, 256] fp8 — two nodes
per 256-byte row) because dma_gather requires 256B-aligned elements. Each
128-slot aggregation tile holds only even-src or only odd-src edges; the
one-hot S matmul for an even tile reads bytes 0:128 of the gathered pair
rows, an odd tile reads bytes 128:256, so every matmul keeps K=128 (partial-
partition matmuls are flaky on HW). Pair row ids all fit int16
(25088 < 32768) so there is no lo/hi index split. Gather calls are capped at
1024 indices — the SWDGE descriptor ring holds 1024 and a larger single call
wedges the device.

The one-hot S (edge-slot -> dst) and Sp (node -> graph) matrices are built
on device by DVE iota-compare from compact bf16 id tables instead of being
shipped as multi-MB inputs and re-streamed from HBM during aggregation.

Host does integer-only graph preprocessing; all floating-point math runs on
device. fp8 quantization noise averages out in aggregation + pooling
(measured ~1e-4 end-to-end rel err vs the fp32 reference; gate is 2e-2).
"""
import numpy as np
import ml_dtypes

N_NODES = 50000
N_EDGES = 600000
HID = 128
OUT_CH = 16
N_GRAPHS = 512
N_CORES = 8
P = 128
SH = N_NODES // N_CORES          # 6250 nodes per shard
NF = (SH + P - 1) // P           # 49 frames of 128 nodes
SHP = NF * P                     # 6272 padded shard rows
NPAIR = SHP // 2                 # 3136 pair rows per shard
NFULL = N_CORES * SHP            # 50176 padded gather-table rows
CF = 6                           # frames per aggregation chunk (PSUM banks)
GT = 8                           # gather tiles per call (1024-desc ring cap)
FQ0 = 25                         # frames in src-chunk 0 (AllGather split)
PQ_SPLIT = FQ0 * (P // 2)        # 1600 pair rows in chunk 0
RQ = (PQ_SPLIT, NPAIR - PQ_SPLIT)  # pair rows per chunk (1600, 1536)
EMPTY_DREL = 200.0               # slot-empty sentinel (never equals 0..127)
EMPTY_GID = 600                # node-pad sentinel (never equals 0..511)

_CACHE = {}
LAST_RESULT = None  # test.py reads exec_time_ns / trace path from here


def _host_prep(edge_index, batch):
    src = np.asarray(edge_index[0], dtype=np.int64)
    dst = np.asarray(edge_index[1], dtype=np.int64)
    batch = np.asarray(batch, dtype=np.int64)
    bf = ml_dtypes.bfloat16

    deg = np.bincount(dst, minlength=N_NODES) + 1  # + self loop

    # padded gather-table row id for each node
    prow = (np.arange(N_NODES) // SH) * SHP + (np.arange(N_NODES) % SH)

    # edges incl. self loops, keyed by (dst core, dst frame, src parity)
    all_src = np.concatenate([src, np.arange(N_NODES)])
    all_dst = np.concatenate([dst, np.arange(N_NODES)])
    core_of = all_dst // SH
    frame_of = (all_dst % SH) // P
    dstrel = (all_dst % SH) % P
    srow = prow[all_src]
    pair = srow // 2
    par = srow % 2

    lp = (srow % SHP) // 2              # local pair row within shard
    qof = (lp >= PQ_SPLIT).astype(np.int64)
    rel = (srow // SHP) * 0  # placeholder
    rel = (srow // SHP) * RQ[0]
    rel = np.where(qof == 0, (srow // SHP) * RQ[0] + lp,
                   (srow // SHP) * RQ[1] + (lp - PQ_SPLIT))

    key = ((core_of * NF + frame_of) * 2 + qof) * 2 + par
    o = np.argsort(key, kind="stable")
    ksort = key[o]
    srt_rel = rel[o]
    srt_drel = dstrel[o]
    cuts = np.searchsorted(ksort, np.arange(N_CORES * NF * 4 + 1))
    cnts = (cuts[1:] - cuts[:-1]).reshape(N_CORES, NF, 2, 2)

    # tiles per (frame, q, parity): uniform across cores (SPMD-identical
    # program); each 128-slot tile holds edges of one (src-chunk, parity)
    t_fqp = np.maximum((cnts.max(axis=0) + P - 1) // P, 1)  # [NF, 2, 2]

    chunks = []
    f = 0
    while f < NF:
        chunks.append(list(range(f, min(f + CF, NF))))
        f += CF

    ntiles_total = int(t_fqp.sum())
    nslots = ntiles_total * P

    idx_all = np.zeros((N_CORES, P, nslots // 16), dtype=np.int16)
    drel_tab = np.full((N_CORES, P, ntiles_total), EMPTY_DREL, dtype=bf)
    tile_base = 0
    chunk_meta = []  # per chunk: (tile_base, frames, frame->{q:(ev,od)})
    for fr in chunks:
        spans = {fi: {} for fi in fr}
        tb = tile_base
        for q in (0, 1):         # q-major: all q0 tiles, then all q1 tiles
            for fi in fr:
                ev = list(range(tb, tb + int(t_fqp[fi, q, 0])))
                tb += int(t_fqp[fi, q, 0])
                od = list(range(tb, tb + int(t_fqp[fi, q, 1])))
                tb += int(t_fqp[fi, q, 1])
                spans[fi][q] = (ev, od)
        chunk_meta.append((tile_base, fr, spans))
        tile_base = tb
    assert tile_base == ntiles_total

    frame_tiles = {}
    for (_, fr, spans) in chunk_meta:
        for fi in fr:
            frame_tiles[fi] = spans[fi]

    for c in range(N_CORES):
        for fi in range(NF):
            for q in (0, 1):
                for half in (0, 1):  # 0 = even srow (bytes 0:128), 1 = odd
                    tiles = frame_tiles[fi][q][half]
                    k = ((c * NF + fi) * 2 + q) * 2 + half
                    e0, e1 = cuts[k], cuts[k + 1]
                    rows = srt_rel[e0:e1]
                    drel = srt_drel[e0:e1]
                    n = e1 - e0
                    assert n <= len(tiles) * P
                    for j in range(n):
                        t = tiles[j // P]
                        e = j % P
                        drel_tab[c, e, t] = drel[j]
                        slot = t * P + e
                        idx_all[c, slot % 16, slot // 16] = rows[j]
    # replicate idx rows 0..15 to the other 7 groups of 16 partitions
    for g in range(1, 8):
        idx_all[:, 16 * g: 16 * (g + 1), :] = idx_all[:, 0:16, :]

    # dinv per shard, [128, NF] (node f*128+s -> [s, f]), pad -> 1.0
    dinv_sh = np.ones((N_CORES, P, NF), dtype=np.float32)
    for c in range(N_CORES):
        d = deg[c * SH:(c + 1) * SH].astype(np.float32)
        dp = np.concatenate([d, np.ones(SHP - SH, np.float32)])
        dinv_sh[c] = (1.0 / np.sqrt(dp)).reshape(NF, P).T

    # graph id per node, [128, NF] int16 (pad -> sentinel; bf16 cannot
    # represent odd ids >= 256)
    gid_tab = np.full((N_CORES, P, NF), EMPTY_GID, dtype=np.int16)
    for c in range(N_CORES):
        b = batch[c * SH:(c + 1) * SH]
        bp = np.concatenate([b, np.full(SHP - SH, EMPTY_GID, np.int64)])
        gid_tab[c] = bp.reshape(NF, P).T.astype(np.int16)

    cnt = np.maximum(np.bincount(batch, minlength=N_GRAPHS), 1)
    invc_t = (1.0 / cnt.astype(np.float32)).reshape(N_GRAPHS // P, P).T

    return dict(idx_all=idx_all, drel_tab=drel_tab, gid_tab=gid_tab,
                dinv_sh=dinv_sh, invc_t=invc_t, frame_tiles=frame_tiles,
                ntiles_total=ntiles_total, chunk_meta=chunk_meta, t_fqp=t_fqp)


def _build_program(prep):
    import os
    import concourse.tile as tile
    from concourse import bacc, mybir
    from concourse.masks import make_identity

    ntiles = prep["ntiles_total"]
    chunk_meta = prep["chunk_meta"]
    ctmax = max(
        sum(len(s[fi][q][0]) + len(s[fi][q][1]) for fi in fr)
        for (_, fr, s) in chunk_meta for q in (0, 1))

    nc = bacc.Bacc("TRN2", target_bir_lowering=False, debug=False,
                   num_devices=N_CORES)
    f32, bf16 = mybir.dt.float32, mybir.dt.bfloat16
    f8 = mybir.dt.float8e4
    i16 = mybir.dt.int16
    AF = mybir.ActivationFunctionType
    OP = mybir.AluOpType

    # ---- IO ----
    xT_in = nc.dram_tensor("xT_sh", [P, SHP], bf16, kind="ExternalInput").ap()
    W1 = nc.dram_tensor("W1", [HID, HID], f32, kind="ExternalInput").ap()
    W2 = nc.dram_tensor("W2", [HID, HID], f32, kind="ExternalInput").ap()
    Wfc = nc.dram_tensor("Wfc", [HID, OUT_CH], f32, kind="ExternalInput").ap()
    b1c = nc.dram_tensor("b1c", [P, 1], f32, kind="ExternalInput").ap()
    b2r = nc.dram_tensor("b2r", [P, HID], f32, kind="ExternalInput").ap()
    bfcr = nc.dram_tensor("bfcr", [P, OUT_CH], f32, kind="ExternalInput").ap()
    idx_in = nc.dram_tensor("idx_in", [P, ntiles * P // 16], i16, kind="ExternalInput").ap()
    drel_in = nc.dram_tensor("drel_in", [P, ntiles], bf16, kind="ExternalInput").ap()
    gid_in = nc.dram_tensor("gid_in", [P, NF], i16, kind="ExternalInput").ap()
    dinv_in = nc.dram_tensor("dinv_in", [P, NF], f32, kind="ExternalInput").ap()
    invc_in = nc.dram_tensor("invc_in", [P, N_GRAPHS // P], f32, kind="ExternalInput").ap()
    out_d = nc.dram_tensor("out", [N_GRAPHS, OUT_CH], f32, kind="ExternalOutput").ap()

    # internal DRAM: pair-packed hs (two fp8 node rows per 256B row);
    # hs_q[l][q] holds the AllGathered src-chunk q (separate tensors so the
    # chunk-0 gathers never wait on the chunk-1 AllGather)
    hs_sh = [nc.dram_tensor(f"hs_sh{l}", [NPAIR, 2 * HID], f8, kind="Internal").ap()
             for l in range(2)]
    hs_q = [[nc.dram_tensor(f"hs_q{l}_{q}", [N_CORES * RQ[q], 2 * HID], f8,
                            kind="Internal").ap() for q in (0, 1)]
            for l in range(2)]
    fc_part = nc.dram_tensor("fc_part", [OUT_CH, N_GRAPHS], f32, kind="Internal").ap()
    fc_full = nc.dram_tensor("fc_full", [OUT_CH, N_GRAPHS], f32, kind="Internal").ap()

    with tile.TileContext(nc, num_cores=N_CORES) as tc:
        with tc.tile_pool(name="const", bufs=1) as cp, \
             tc.tile_pool(name="persist", bufs=1) as pp, \
             tc.tile_pool(name="work", bufs=3) as wp, \
             tc.tile_pool(name="msgs", bufs=2) as mp, \
             tc.tile_pool(name="psAcc", bufs=6, space="PSUM") as psAcc, \
             tc.tile_pool(name="psX", bufs=2, space="PSUM") as psX, \
             tc.tile_pool(name="dram", bufs=2, space="DRAM") as dp:

            # ---- constants ----
            ident = cp.tile([P, P], f32)
            make_identity(nc, ident[:])
            W1b = cp.tile([P, HID], bf16)
            W2b = cp.tile([P, HID], bf16)
            Wfb = cp.tile([P, OUT_CH], bf16)
            for Wd, Wb in ((W1, W1b), (W2, W2b), (Wfc, Wfb)):
                wf = wp.tile([P, Wd.shape[1]], f32, tag="wtmp")
                nc.sync.dma_start(wf[:], Wd[:])
                nc.vector.tensor_copy(Wb[:], wf[:])
            b1_sb = cp.tile([P, 1], f32)
            nc.sync.dma_start(b1_sb[:], b1c[:])
            b2_sb = cp.tile([P, HID], f32)
            nc.sync.dma_start(b2_sb[:], b2r[:])
            bfc_sb = cp.tile([P, OUT_CH], f32)
            nc.sync.dma_start(bfc_sb[:], bfcr[:])
            # x shard, channel-major (pre-transposed on host)
            xt_sb = pp.tile([P, SHP], bf16)
            nc.sync.dma_start(xt_sb[:], xT_in[:])
            dinv = cp.tile([P, NF], f32)
            nc.sync.dma_start(dinv[:], dinv_in[:])
            invc = cp.tile([P, N_GRAPHS // P], f32)
            nc.sync.dma_start(invc[:], invc_in[:])
            # id tables + the whole gather-index table (resident in SBUF)
            drel_sb = cp.tile([P, ntiles], bf16)
            nc.sync.dma_start(drel_sb[:], drel_in[:])
            gid_sb = cp.tile([P, NF], i16)
            nc.sync.dma_start(gid_sb[:], gid_in[:])
            idx_sb = pp.tile([P, ntiles * P // 16], i16)
            nc.sync.dma_start(idx_sb[:], idx_in[:])
            # iota ramps for the one-hot builds
            iota_rep = cp.tile([P, ctmax, P], bf16)  # 0..127 per tile
            nc.gpsimd.iota(iota_rep[:], pattern=[[0, ctmax], [1, P]], base=0,
                           channel_multiplier=0,
                           allow_small_or_imprecise_dtypes=True)
            iota_g = cp.tile([P, N_GRAPHS], i16)     # 0..511
            nc.gpsimd.iota(iota_g[:], pattern=[[1, N_GRAPHS]], base=0,
                           channel_multiplier=0,
                           allow_small_or_imprecise_dtypes=True)

            dinv_b = pp.tile([P, NF, HID], bf16)  # dinv broadcast per frame
            nc.vector.tensor_copy(
                dinv_b[:], dinv[:].unsqueeze(2).broadcast_to([P, NF, HID]))

            out1T = pp.tile([P, SHP], bf16)   # layer-1 output, ch-major
            out2 = pp.tile([P, NF, HID], bf16)  # layer-2 output, node-major

            # ---- layer matmul stages ----
            # 4 frames share one PSUM bank + one fused DVE scale + one DMA
            # (per-frame sync chains dominated the stage otherwise)
            def matmul_stage(layer, f0, f1):
                for b0 in range(f0, f1, 4):
                    b1 = min(b0 + 4, f1)
                    F = b1 - b0
                    u_ps = psX.tile([P, F, HID], f32, space="PSUM", tag="mm",
                                    name=f"u{layer}_{b0}")
                    for b in range(b0, b1):
                        lhs_ap = (xt_sb if layer == 0 else out1T)[:, b * P:(b + 1) * P]
                        Wb = W1b if layer == 0 else W2b
                        nc.tensor.matmul(u_ps[:, b - b0, :], lhsT=lhs_ap,
                                         rhs=Wb[:], start=True, stop=True,
                                         skip_group_check=True)
                    hsb = wp.tile([P, F, HID], f8, tag="hsb")
                    nc.vector.tensor_tensor(hsb[:], u_ps[:],
                                            dinv_b[:, b0:b1, :], op=OP.mult)
                    # pair rows [b0*64, b1*64); dram order is node-major,
                    # sbuf is partition-major -> reorder via the dst AP
                    dst = hs_sh[layer][b0 * (P // 2):b1 * (P // 2), :]
                    dst = dst.rearrange("(f n2) (two h) -> (n2 two) f h",
                                        f=F, two=2)
                    nc.sync.dma_start(dst, hsb[:])

            def allgather(layer, q):
                p0 = 0 if q == 0 else PQ_SPLIT
                nc.gpsimd.collective_compute(
                    "AllGather", OP.bypass,
                    replica_groups=[list(range(N_CORES))],
                    ins=[hs_sh[layer][p0:p0 + RQ[q], :]], outs=[hs_q[layer][q]],
                )

            # ---- aggregation stage ----
            # f32 accumulators for the two-pass aggregation (q0 sums
            # buffered here while the q1 AllGather is still in flight)
            acc_all = pp.tile([P, NF, HID], f32)

            def agg_pass(layer, q):
                k_agg = os.environ.get("K_AGG", "")
                src = hs_q[layer][q]
                for (tb, fr, spans) in chunk_meta:
                    qt = [t for fi in fr for t in spans[fi][q][0] + spans[fi][q][1]]
                    q0, q1t = min(qt), max(qt) + 1  # q-tiles are contiguous
                    ct = q1t - q0
                    msg = mp.tile([P, ct, 2 * HID], f8, tag="msg")
                    # one-hot S for this chunk's q-tiles, built on DVE
                    s_sb = mp.tile([P, ct, P], f8, tag="S")
                    nc.vector.tensor_tensor(
                        s_sb[:],
                        drel_sb[:, q0:q1t].unsqueeze(2).broadcast_to([P, ct, P]),
                        iota_rep[:, 0:ct, :],
                        op=OP.is_equal)
                    if k_agg == "dma":
                        continue
                    # gather in <=1024-idx calls (SWDGE ring capacity)
                    for g0 in range(0, ct, GT):
                        g1 = min(g0 + GT, ct)
                        nc.gpsimd.dma_gather(
                            out_ap=msg[:, g0:g1, :], in_ap=src[:],
                            idxs_ap=idx_sb[:, (q0 + g0) * P // 16:(q0 + g1) * P // 16],
                            num_idxs=(g1 - g0) * P, num_idxs_reg=(g1 - g0) * P,
                            elem_size=2 * HID)
                    if k_agg == "gather":
                        continue
                    accs = {}
                    for fi in fr:
                        accs[fi] = psAcc.tile([P, HID], f32, space="PSUM", tag="acc", name=f"acc{layer}_{q}_{fi}")
                    # absorber: single dummy matmul observes S + msg + acc sems
                    nc.tensor.matmul(accs[fr[0]][0:2, 0:2], lhsT=s_sb[:, 0, 0:2],
                                     rhs=msg[:, 0, 0:2], start=True, stop=True,
                                     skip_group_check=True)
                    # per frame: one K=128 matmul per tile; even-src tiles
                    # read bytes 0:128 of the gathered pair row, odd 128:256
                    for fi in fr:
                        ev, od = spans[fi][q]
                        nmm = len(ev) + len(od)
                        j = 0
                        for t, c0 in [(t, 0) for t in ev] + [(t, HID) for t in od]:
                            tl = t - q0
                            nc.tensor.matmul(
                                accs[fi][:],
                                lhsT=s_sb[:, tl, :],
                                rhs=msg[:, tl, c0:c0 + HID],
                                start=(j == 0), stop=(j == nmm - 1),
                                skip_group_check=True)
                            j += 1
                    if k_agg == "mm":
                        continue
                    for fi in fr:
                        if q == 0:
                            nc.vector.tensor_copy(acc_all[:, fi, :], accs[fi][:])
                            continue
                        ag = wp.tile([P, HID], f32, tag="ag")
                        nc.vector.tensor_tensor(ag[:], accs[fi][:],
                                                acc_all[:, fi, :], op=OP.add)
                        ags = wp.tile([P, HID], f32, tag="ags")
                        nc.vector.tensor_scalar(ags[:], ag[:],
                                                dinv[:, fi:fi + 1], None, OP.mult)
                        if layer == 0:
                            agT = psX.tile([P, P], f32, space="PSUM", tag="mm", name=f"agT{fi}")
                            nc.tensor.transpose(agT[:], ags[:], ident[:])
                            nc.scalar.activation(
                                out1T[:, fi * P:(fi + 1) * P], agT[:],
                                AF.Relu, bias=b1_sb[:, 0:1])
                        else:
                            ab = wp.tile([P, HID], f32, tag="ab")
                            nc.vector.tensor_tensor(ab[:], ags[:], b2_sb[:],
                                                    op=OP.add)
                            nc.scalar.activation(out2[:, fi, :], ab[:], AF.Relu)

            # ---- pooling + FC (FC partials AllReduced, [16, 512] f32) ----
            def pool_fc():
                pl_ps = psX.tile([P, N_GRAPHS], f32, space="PSUM", tag="mm", name="pl_ps")
                nc.tensor.matmul(pl_ps[0:1, 0:1], lhsT=out2[:, 0, 0:1],
                                 rhs=out2[:, 0, 0:1], start=True, stop=True,
                                 skip_group_check=True)
                for f in range(NF):
                    sp = wp.tile([P, N_GRAPHS], f8, tag="sp")
                    nc.vector.tensor_tensor(
                        sp[:],
                        gid_sb[:, f:f + 1].broadcast_to([P, N_GRAPHS]),
                        iota_g[:], op=OP.is_equal)
                    nc.tensor.matmul(pl_ps[:], lhsT=out2[:, f, :], rhs=sp[:],
                                     start=(f == 0), stop=(f == NF - 1),
                                     skip_group_check=True)
                pf = wp.tile([P, N_GRAPHS], bf16, tag="plsb")
                nc.vector.tensor_copy(pf[:], pl_ps[:])
                fc_ps = psX.tile([OUT_CH, N_GRAPHS], f32, space="PSUM", tag="mm", name="fc_ps")
                nc.tensor.matmul(fc_ps[:], lhsT=Wfb[:], rhs=pf[:],
                                 start=True, stop=True)
                fcp = wp.tile([OUT_CH, N_GRAPHS], f32, tag="fcp")
                nc.vector.tensor_copy(fcp[:], fc_ps[:])
                nc.sync.dma_start(fc_part[:], fcp[:])
                nc.gpsimd.collective_compute(
                    "AllReduce", OP.add, replica_groups=[list(range(N_CORES))],
                    ins=[fc_part], outs=[fc_full])
                fcT = pp.tile([OUT_CH, N_GRAPHS], f32)
                nc.sync.dma_start(fcT[:], fc_full[:])
                for b in range(N_GRAPHS // P):
                    tb_ps = psX.tile([P, OUT_CH], f32, space="PSUM", tag="mm", name=f"tbp{b}")
                    nc.tensor.matmul(tb_ps[:], lhsT=fcT[:, b * P:(b + 1) * P],
                                     rhs=ident[:OUT_CH, :OUT_CH],
                                     is_transpose=True, start=True, stop=True)
                    sc = wp.tile([P, OUT_CH], f32, tag="sc")
                    nc.vector.tensor_scalar(sc[:], tb_ps[:], invc[:, b:b + 1],
                                            None, OP.mult)
                    ad = wp.tile([P, OUT_CH], f32, tag="ad")
                    nc.vector.tensor_tensor(ad[:], sc[:], bfc_sb[:], op=OP.add)
                    sg = wp.tile([P, OUT_CH], f32, tag="sg")
                    nc.scalar.activation(sg[:], ad[:], AF.Sigmoid)
                    nc.sync.dma_start(out_d[b * P:(b + 1) * P, :], sg[:])

            stage_limit = int(os.environ.get("K_STAGE", "0"))

            def dbg_out():
                for b in range(N_GRAPHS // P):
                    t = wp.tile([P, OUT_CH], f32, tag="dbg", name=f"dbg{b}")
                    nc.vector.tensor_copy(t[:], b2_sb[:, 0:OUT_CH])
                    nc.sync.dma_start(out_d[b * P:(b + 1) * P, :], t[:])

            stages = [
                lambda: matmul_stage(0, 0, FQ0),
                lambda: allgather(0, 0),
                lambda: matmul_stage(0, FQ0, NF),
                lambda: allgather(0, 1),
                lambda: agg_pass(0, 0),
                lambda: agg_pass(0, 1),
                lambda: matmul_stage(1, 0, FQ0),
                lambda: allgather(1, 0),
                lambda: matmul_stage(1, FQ0, NF),
                lambda: allgather(1, 1),
                lambda: agg_pass(1, 0),
                lambda: agg_pass(1, 1),
                pool_fc,
            ]
            nstage = stage_limit if stage_limit > 0 else len(stages)
            for s in stages[:nstage]:
                s()
            if nstage < len(stages):
                dbg_out()

    nc.compile()
    return nc


def _in_maps_for(prep, x, W1, b1, W2, b2, Wfc, bfc):
    bf = ml_dtypes.bfloat16
    xT = np.zeros((N_CORES, P, SHP), dtype=bf)
    for c in range(N_CORES):
        xT[c, :, :SH] = x[c * SH:(c + 1) * SH].T
    b1c = b1.reshape(P, 1)
    b2r = np.broadcast_to(b2.reshape(1, HID), (P, HID)).copy()
    bfcr = np.broadcast_to(bfc.reshape(1, OUT_CH), (P, OUT_CH)).copy()
    in_maps = []
    for c in range(N_CORES):
        in_maps.append({
            "xT_sh": xT[c], "W1": W1, "W2": W2, "Wfc": Wfc,
            "b1c": b1c, "b2r": b2r, "bfcr": bfcr,
            "idx_in": np.ascontiguousarray(prep["idx_all"][c]),
            "drel_in": np.ascontiguousarray(prep["drel_tab"][c]),
            "gid_in": np.ascontiguousarray(prep["gid_tab"][c]),
            "dinv_in": np.ascontiguousarray(prep["dinv_sh"][c]),
            "invc_in": np.ascontiguousarray(prep["invc_t"]),
        })
    return in_maps


def kernel(x, edge_index, batch, W1, b1, W2, b2, Wfc, bfc):
    from concourse.bass_utils import run_bass_kernel_spmd

    x = np.asarray(x, dtype=np.float32)
    W1 = np.asarray(W1, dtype=np.float32)
    W2 = np.asarray(W2, dtype=np.float32)
    Wfc = np.asarray(Wfc, dtype=np.float32)
    b1 = np.asarray(b1, dtype=np.float32)
    b2 = np.asarray(b2, dtype=np.float32)
    bfc = np.asarray(bfc, dtype=np.float32)

    key = (int(np.asarray(edge_index).sum()) & 0xFFFFFFFF,)
    if key not in _CACHE:
        prep = _host_prep(edge_index, batch)
        prog = _build_program(prep)
        _CACHE[key] = (prep, prog)
    prep, prog = _CACHE[key]

    in_maps = _in_maps_for(prep, x, W1, b1, W2, b2, Wfc, bfc)
    global LAST_RESULT
    res = run_bass_kernel_spmd(prog, in_maps, core_ids=list(range(N_CORES)))
    LAST_RESULT = res
    return np.asarray(res.results[0]["out"], dtype=np.float32)


# revision 30
# speedup vs baseline: 1.1777x; 1.1777x over previous
"""GCN (2x GCNConv + global_mean_pool + FC + sigmoid) on 8 TRN2 NeuronCores.

Sharding: nodes (and incident edges, by dst) are partitioned across 8 cores.
Each core computes the feature transform + message aggregation for its 6250
dst nodes; hs (dinv-scaled transformed features, fp8-e4m3) is AllGathered
between layers; the [16, 512] FC partials are AllReduced at the end.

fp8 message path: hs rows are stored pair-packed ([n/2, 256] fp8 — two nodes
per 256-byte row) because dma_gather requires 256B-aligned elements. Each
128-slot aggregation tile holds only even-src or only odd-src edges; the
one-hot S matmul for an even tile reads bytes 0:128 of the gathered pair
rows, an odd tile reads bytes 128:256, so every matmul keeps K=128 (partial-
partition matmuls are flaky on HW). Pair row ids all fit int16
(25088 < 32768) so there is no lo/hi index split. Gather calls are capped at
1024 indices — the SWDGE descriptor ring holds 1024 and a larger single call
wedges the device.

The one-hot S (edge-slot -> dst) and Sp (node -> graph) matrices are built
on device by DVE iota-compare from compact bf16 id tables instead of being
shipped as multi-MB inputs and re-streamed from HBM during aggregation.

Host does integer-only graph preprocessing; all floating-point math runs on
device. fp8 quantization noise averages out in aggregation + pooling
(measured ~1e-4 end-to-end rel err vs the fp32 reference; gate is 2e-2).
"""
import numpy as np
import ml_dtypes

N_NODES = 50000
N_EDGES = 600000
HID = 128
OUT_CH = 16
N_GRAPHS = 512
N_CORES = 8
P = 128
SH = N_NODES // N_CORES          # 6250 nodes per shard
NF = (SH + P - 1) // P           # 49 frames of 128 nodes
SHP = NF * P                     # 6272 padded shard rows
NPAIR = SHP // 2                 # 3136 pair rows per shard
NFULL = N_CORES * SHP            # 50176 padded gather-table rows
CF = 6                           # frames per aggregation chunk (PSUM banks)
GT = 8                           # gather tiles per call (1024-desc ring cap)
FQ0 = 25                         # frames in src-chunk 0 (AllGather split)
PQ_SPLIT = FQ0 * (P // 2)        # 1600 pair rows in chunk 0
RQ = (PQ_SPLIT, NPAIR - PQ_SPLIT)  # pair rows per chunk (1600, 1536)
EMPTY_DREL = 200.0               # slot-empty sentinel (never equals 0..127)
EMPTY_GID = 600                # node-pad sentinel (never equals 0..511)

_CACHE = {}
LAST_RESULT = None  # test.py reads exec_time_ns / trace path from here


def _host_prep(edge_index, batch):
    src = np.asarray(edge_index[0], dtype=np.int64)
    dst = np.asarray(edge_index[1], dtype=np.int64)
    batch = np.asarray(batch, dtype=np.int64)
    bf = ml_dtypes.bfloat16

    deg = np.bincount(dst, minlength=N_NODES) + 1  # + self loop

    # padded gather-table row id for each node
    prow = (np.arange(N_NODES) // SH) * SHP + (np.arange(N_NODES) % SH)

    # edges incl. self loops, keyed by (dst core, dst frame, src parity)
    all_src = np.concatenate([src, np.arange(N_NODES)])
    all_dst = np.concatenate([dst, np.arange(N_NODES)])
    core_of = all_dst // SH
    frame_of = (all_dst % SH) // P
    dstrel = (all_dst % SH) % P
    srow = prow[all_src]
    pair = srow // 2
    par = srow % 2

    lp = (srow % SHP) // 2              # local pair row within shard
    qof = (lp >= PQ_SPLIT).astype(np.int64)
    rel = (srow // SHP) * 0  # placeholder
    rel = (srow // SHP) * RQ[0]
    rel = np.where(qof == 0, (srow // SHP) * RQ[0] + lp,
                   (srow // SHP) * RQ[1] + (lp - PQ_SPLIT))

    key = ((core_of * NF + frame_of) * 2 + qof) * 2 + par
    o = np.argsort(key, kind="stable")
    ksort = key[o]
    srt_rel = rel[o]
    srt_drel = dstrel[o]
    cuts = np.searchsorted(ksort, np.arange(N_CORES * NF * 4 + 1))
    cnts = (cuts[1:] - cuts[:-1]).reshape(N_CORES, NF, 2, 2)

    # tiles per (frame, q, parity): uniform across cores (SPMD-identical
    # program); each 128-slot tile holds edges of one (src-chunk, parity)
    t_fqp = np.maximum((cnts.max(axis=0) + P - 1) // P, 1)  # [NF, 2, 2]

    chunks = []
    f = 0
    while f < NF:
        chunks.append(list(range(f, min(f + CF, NF))))
        f += CF

    ntiles_total = int(t_fqp.sum())
    nslots = ntiles_total * P

    idx_all = np.zeros((N_CORES, P, nslots // 16), dtype=np.int16)
    drel_tab = np.full((N_CORES, P, ntiles_total), EMPTY_DREL, dtype=bf)
    tile_base = 0
    chunk_meta = []  # per chunk: (tile_base, frames, frame->{q:(ev,od)})
    for fr in chunks:
        spans = {fi: {} for fi in fr}
        tb = tile_base
        for q in (0, 1):         # q-major: all q0 tiles, then all q1 tiles
            for fi in fr:
                ev = list(range(tb, tb + int(t_fqp[fi, q, 0])))
                tb += int(t_fqp[fi, q, 0])
                od = list(range(tb, tb + int(t_fqp[fi, q, 1])))
                tb += int(t_fqp[fi, q, 1])
                spans[fi][q] = (ev, od)
        chunk_meta.append((tile_base, fr, spans))
        tile_base = tb
    assert tile_base == ntiles_total

    frame_tiles = {}
    for (_, fr, spans) in chunk_meta:
        for fi in fr:
            frame_tiles[fi] = spans[fi]

    for c in range(N_CORES):
        for fi in range(NF):
            for q in (0, 1):
                for half in (0, 1):  # 0 = even srow (bytes 0:128), 1 = odd
                    tiles = frame_tiles[fi][q][half]
                    k = ((c * NF + fi) * 2 + q) * 2 + half
                    e0, e1 = cuts[k], cuts[k + 1]
                    rows = srt_rel[e0:e1]
                    drel = srt_drel[e0:e1]
                    n = e1 - e0
                    assert n <= len(tiles) * P
                    for j in range(n):
                        t = tiles[j // P]
                        e = j % P
                        drel_tab[c, e, t] = drel[j]
                        slot = t * P + e
                        idx_all[c, slot % 16, slot // 16] = rows[j]
    # replicate idx rows 0..15 to the other 7 groups of 16 partitions
    for g in range(1, 8):
        idx_all[:, 16 * g: 16 * (g + 1), :] = idx_all[:, 0:16, :]

    # dinv per shard, [128, NF] (node f*128+s -> [s, f]), pad -> 1.0
    dinv_sh = np.ones((N_CORES, P, NF), dtype=np.float32)
    for c in range(N_CORES):
        d = deg[c * SH:(c + 1) * SH].astype(np.float32)
        dp = np.concatenate([d, np.ones(SHP - SH, np.float32)])
        dinv_sh[c] = (1.0 / np.sqrt(dp)).reshape(NF, P).T

    # graph id per node, [128, NF] int16 (pad -> sentinel; bf16 cannot
    # represent odd ids >= 256)
    gid_tab = np.full((N_CORES, P, NF), EMPTY_GID, dtype=np.int16)
    for c in range(N_CORES):
        b = batch[c * SH:(c + 1) * SH]
        bp = np.concatenate([b, np.full(SHP - SH, EMPTY_GID, np.int64)])
        gid_tab[c] = bp.reshape(NF, P).T.astype(np.int16)

    cnt = np.maximum(np.bincount(batch, minlength=N_GRAPHS), 1)
    invc_t = (1.0 / cnt.astype(np.float32)).reshape(N_GRAPHS // P, P).T

    return dict(idx_all=idx_all, drel_tab=drel_tab, gid_tab=gid_tab,
                dinv_sh=dinv_sh, invc_t=invc_t, frame_tiles=frame_tiles,
                ntiles_total=ntiles_total, chunk_meta=chunk_meta, t_fqp=t_fqp)


def _build_program(prep):
    import os
    import concourse.tile as tile
    from concourse import bacc, mybir
    from concourse.masks import make_identity

    ntiles = prep["ntiles_total"]
    chunk_meta = prep["chunk_meta"]
    ctmax = max(
        sum(len(s[fi][q][0]) + len(s[fi][q][1]) for fi in fr)
        for (_, fr, s) in chunk_meta for q in (0, 1))

    nc = bacc.Bacc("TRN2", target_bir_lowering=False, debug=False,
                   num_devices=N_CORES)
    f32, bf16 = mybir.dt.float32, mybir.dt.bfloat16
    f8 = mybir.dt.float8e4
    i16 = mybir.dt.int16
    AF = mybir.ActivationFunctionType
    OP = mybir.AluOpType

    # ---- IO ----
    xT_in = nc.dram_tensor("xT_sh", [P, SHP], bf16, kind="ExternalInput").ap()
    W1 = nc.dram_tensor("W1", [HID, HID], f32, kind="ExternalInput").ap()
    W2 = nc.dram_tensor("W2", [HID, HID], f32, kind="ExternalInput").ap()
    Wfc = nc.dram_tensor("Wfc", [HID, OUT_CH], f32, kind="ExternalInput").ap()
    b1c = nc.dram_tensor("b1c", [P, 1], f32, kind="ExternalInput").ap()
    b2r = nc.dram_tensor("b2r", [P, HID], f32, kind="ExternalInput").ap()
    bfcr = nc.dram_tensor("bfcr", [P, OUT_CH], f32, kind="ExternalInput").ap()
    idx_in = nc.dram_tensor("idx_in", [P, ntiles * P // 16], i16, kind="ExternalInput").ap()
    drel_in = nc.dram_tensor("drel_in", [P, ntiles], bf16, kind="ExternalInput").ap()
    gid_in = nc.dram_tensor("gid_in", [P, NF], i16, kind="ExternalInput").ap()
    dinv_in = nc.dram_tensor("dinv_in", [P, NF], f32, kind="ExternalInput").ap()
    invc_in = nc.dram_tensor("invc_in", [P, N_GRAPHS // P], f32, kind="ExternalInput").ap()
    out_d = nc.dram_tensor("out", [N_GRAPHS, OUT_CH], f32, kind="ExternalOutput").ap()

    # internal DRAM: pair-packed hs (two fp8 node rows per 256B row);
    # hs_q[l][q] holds the AllGathered src-chunk q (separate tensors so the
    # chunk-0 gathers never wait on the chunk-1 AllGather)
    hs_sh = [nc.dram_tensor(f"hs_sh{l}", [NPAIR, 2 * HID], f8, kind="Internal").ap()
             for l in range(2)]
    hs_q = [[nc.dram_tensor(f"hs_q{l}_{q}", [N_CORES * RQ[q], 2 * HID], f8,
                            kind="Internal").ap() for q in (0, 1)]
            for l in range(2)]
    fc_part = nc.dram_tensor("fc_part", [OUT_CH, N_GRAPHS], f32, kind="Internal").ap()
    fc_full = nc.dram_tensor("fc_full", [OUT_CH, N_GRAPHS], f32, kind="Internal").ap()

    with tile.TileContext(nc, num_cores=N_CORES) as tc:
        with tc.tile_pool(name="const", bufs=1) as cp, \
             tc.tile_pool(name="persist", bufs=1) as pp, \
             tc.tile_pool(name="work", bufs=3) as wp, \
             tc.tile_pool(name="msgs", bufs=2) as mp, \
             tc.tile_pool(name="psAcc", bufs=6, space="PSUM") as psAcc, \
             tc.tile_pool(name="psX", bufs=2, space="PSUM") as psX, \
             tc.tile_pool(name="dram", bufs=2, space="DRAM") as dp:

            # ---- constants ----
            ident = cp.tile([P, P], f32)
            make_identity(nc, ident[:])
            W1b = cp.tile([P, HID], bf16)
            W2b = cp.tile([P, HID], bf16)
            Wfb = cp.tile([P, OUT_CH], bf16)
            for Wd, Wb in ((W1, W1b), (W2, W2b), (Wfc, Wfb)):
                wf = wp.tile([P, Wd.shape[1]], f32, tag="wtmp")
                nc.sync.dma_start(wf[:], Wd[:])
                nc.vector.tensor_copy(Wb[:], wf[:])
            b1_sb = cp.tile([P, 1], f32)
            nc.sync.dma_start(b1_sb[:], b1c[:])
            b2_sb = cp.tile([P, HID], f32)
            nc.sync.dma_start(b2_sb[:], b2r[:])
            bfc_sb = cp.tile([P, OUT_CH], f32)
            nc.sync.dma_start(bfc_sb[:], bfcr[:])
            # x shard, channel-major (pre-transposed on host)
            xt_sb = pp.tile([P, SHP], bf16)
            nc.sync.dma_start(xt_sb[:], xT_in[:])
            dinv = cp.tile([P, NF], f32)
            nc.sync.dma_start(dinv[:], dinv_in[:])
            invc = cp.tile([P, N_GRAPHS // P], f32)
            nc.sync.dma_start(invc[:], invc_in[:])
            # id tables + the whole gather-index table (resident in SBUF)
            drel_sb = cp.tile([P, ntiles], bf16)
            nc.sync.dma_start(drel_sb[:], drel_in[:])
            gid_sb = cp.tile([P, NF], i16)
            nc.sync.dma_start(gid_sb[:], gid_in[:])
            idx_sb = pp.tile([P, ntiles * P // 16], i16)
            nc.sync.dma_start(idx_sb[:], idx_in[:])
            # iota ramps for the one-hot builds
            iota_rep = cp.tile([P, ctmax, P], bf16)  # 0..127 per tile
            nc.gpsimd.iota(iota_rep[:], pattern=[[0, ctmax], [1, P]], base=0,
                           channel_multiplier=0,
                           allow_small_or_imprecise_dtypes=True)
            iota_g = cp.tile([P, N_GRAPHS], i16)     # 0..511
            nc.gpsimd.iota(iota_g[:], pattern=[[1, N_GRAPHS]], base=0,
                           channel_multiplier=0,
                           allow_small_or_imprecise_dtypes=True)

            sp_all = pp.tile([P, NF, N_GRAPHS], f8)  # pooling one-hots

            def build_sp():
                # emitted after the first AllGather so the DVE work lands in
                # the collective's idle window, not ahead of the mm scales
                for f in range(NF):
                    nc.vector.tensor_tensor(
                        sp_all[:, f, :],
                        gid_sb[:, f:f + 1].broadcast_to([P, N_GRAPHS]),
                        iota_g[:], op=OP.is_equal)

            dinv_b = pp.tile([P, NF, HID], bf16)  # dinv broadcast per frame
            nc.vector.tensor_copy(
                dinv_b[:], dinv[:].unsqueeze(2).broadcast_to([P, NF, HID]))

            out1T = pp.tile([P, SHP], bf16)   # layer-1 output, ch-major
            out2 = pp.tile([P, NF, HID], bf16)  # layer-2 output, node-major

            # ---- layer matmul stages ----
            # 4 frames share one PSUM bank + one fused DVE scale + one DMA
            # (per-frame sync chains dominated the stage otherwise)
            def matmul_stage(layer, f0, f1):
                for b0 in range(f0, f1, 4):
                    b1 = min(b0 + 4, f1)
                    F = b1 - b0
                    u_ps = psX.tile([P, F, HID], f32, space="PSUM", tag="mm",
                                    name=f"u{layer}_{b0}")
                    for b in range(b0, b1):
                        lhs_ap = (xt_sb if layer == 0 else out1T)[:, b * P:(b + 1) * P]
                        Wb = W1b if layer == 0 else W2b
                        nc.tensor.matmul(u_ps[:, b - b0, :], lhsT=lhs_ap,
                                         rhs=Wb[:], start=True, stop=True,
                                         skip_group_check=True)
                    hsb = wp.tile([P, F, HID], f8, tag="hsb")
                    nc.vector.tensor_tensor(hsb[:], u_ps[:],
                                            dinv_b[:, b0:b1, :], op=OP.mult)
                    # pair rows [b0*64, b1*64); dram order is node-major,
                    # sbuf is partition-major -> reorder via the dst AP
                    dst = hs_sh[layer][b0 * (P // 2):b1 * (P // 2), :]
                    dst = dst.rearrange("(f n2) (two h) -> (n2 two) f h",
                                        f=F, two=2)
                    nc.sync.dma_start(dst, hsb[:])

            def allgather(layer, q):
                p0 = 0 if q == 0 else PQ_SPLIT
                nc.gpsimd.collective_compute(
                    "AllGather", OP.bypass,
                    replica_groups=[list(range(N_CORES))],
                    ins=[hs_sh[layer][p0:p0 + RQ[q], :]], outs=[hs_q[layer][q]],
                )

            # ---- aggregation stage ----
            # f32 accumulators for the two-pass aggregation (q0 sums
            # buffered here while the q1 AllGather is still in flight)
            acc_all = pp.tile([P, NF, HID], f32)

            def agg_pass(layer, q):
                k_agg = os.environ.get("K_AGG", "")
                src = hs_q[layer][q]
                for (tb, fr, spans) in chunk_meta:
                    qt = [t for fi in fr for t in spans[fi][q][0] + spans[fi][q][1]]
                    q0, q1t = min(qt), max(qt) + 1  # q-tiles are contiguous
                    ct = q1t - q0
                    msg = mp.tile([P, ct, 2 * HID], f8, tag="msg")
                    # one-hot S for this chunk's q-tiles, built on DVE
                    s_sb = mp.tile([P, ct, P], f8, tag="S")
                    nc.vector.tensor_tensor(
                        s_sb[:],
                        drel_sb[:, q0:q1t].unsqueeze(2).broadcast_to([P, ct, P]),
                        iota_rep[:, 0:ct, :],
                        op=OP.is_equal)
                    if k_agg == "dma":
                        continue
                    # gather in <=1024-idx calls (SWDGE ring capacity)
                    for g0 in range(0, ct, GT):
                        g1 = min(g0 + GT, ct)
                        nc.gpsimd.dma_gather(
                            out_ap=msg[:, g0:g1, :], in_ap=src[:],
                            idxs_ap=idx_sb[:, (q0 + g0) * P // 16:(q0 + g1) * P // 16],
                            num_idxs=(g1 - g0) * P, num_idxs_reg=(g1 - g0) * P,
                            elem_size=2 * HID)
                    if k_agg == "gather":
                        continue
                    accs = {}
                    for fi in fr:
                        accs[fi] = psAcc.tile([P, HID], f32, space="PSUM", tag="acc", name=f"acc{layer}_{q}_{fi}")
                    # absorber: single dummy matmul observes S + msg + acc sems
                    nc.tensor.matmul(accs[fr[0]][0:2, 0:2], lhsT=s_sb[:, 0, 0:2],
                                     rhs=msg[:, 0, 0:2], start=True, stop=True,
                                     skip_group_check=True)
                    # per frame: one K=128 matmul per tile; even-src tiles
                    # read bytes 0:128 of the gathered pair row, odd 128:256
                    for fi in fr:
                        ev, od = spans[fi][q]
                        nmm = len(ev) + len(od)
                        j = 0
                        for t, c0 in [(t, 0) for t in ev] + [(t, HID) for t in od]:
                            tl = t - q0
                            nc.tensor.matmul(
                                accs[fi][:],
                                lhsT=s_sb[:, tl, :],
                                rhs=msg[:, tl, c0:c0 + HID],
                                start=(j == 0), stop=(j == nmm - 1),
                                skip_group_check=True)
                            j += 1
                    if k_agg == "mm":
                        continue
                    for fi in fr:
                        if q == 0:
                            nc.vector.tensor_copy(acc_all[:, fi, :], accs[fi][:])
                            continue
                        ag = wp.tile([P, HID], f32, tag="ag")
                        nc.vector.tensor_tensor(ag[:], accs[fi][:],
                                                acc_all[:, fi, :], op=OP.add)
                        ags = wp.tile([P, HID], f32, tag="ags")
                        nc.vector.tensor_scalar(ags[:], ag[:],
                                                dinv[:, fi:fi + 1], None, OP.mult)
                        if layer == 0:
                            agT = psX.tile([P, P], f32, space="PSUM", tag="mm", name=f"agT{fi}")
                            nc.tensor.transpose(agT[:], ags[:], ident[:])
                            nc.scalar.activation(
                                out1T[:, fi * P:(fi + 1) * P], agT[:],
                                AF.Relu, bias=b1_sb[:, 0:1])
                        else:
                            ab = wp.tile([P, HID], f32, tag="ab")
                            nc.vector.tensor_tensor(ab[:], ags[:], b2_sb[:],
                                                    op=OP.add)
                            nc.scalar.activation(out2[:, fi, :], ab[:], AF.Relu)

            # ---- pooling + FC (FC partials AllReduced, [16, 512] f32) ----
            def pool_fc():
                pl_ps = psX.tile([P, N_GRAPHS], f32, space="PSUM", tag="mm", name="pl_ps")
                nc.tensor.matmul(pl_ps[0:1, 0:1], lhsT=out2[:, 0, 0:1],
                                 rhs=out2[:, 0, 0:1], start=True, stop=True,
                                 skip_group_check=True)
                for f in range(NF):
                    nc.tensor.matmul(pl_ps[:], lhsT=out2[:, f, :],
                                     rhs=sp_all[:, f, :],
                                     start=(f == 0), stop=(f == NF - 1),
                                     skip_group_check=True)
                pf = wp.tile([P, N_GRAPHS], bf16, tag="plsb")
                nc.vector.tensor_copy(pf[:], pl_ps[:])
                fc_ps = psX.tile([OUT_CH, N_GRAPHS], f32, space="PSUM", tag="mm", name="fc_ps")
                nc.tensor.matmul(fc_ps[:], lhsT=Wfb[:], rhs=pf[:],
                                 start=True, stop=True)
                fcp = wp.tile([OUT_CH, N_GRAPHS], f32, tag="fcp")
                nc.vector.tensor_copy(fcp[:], fc_ps[:])
                nc.sync.dma_start(fc_part[:], fcp[:])
                nc.gpsimd.collective_compute(
                    "AllReduce", OP.add, replica_groups=[list(range(N_CORES))],
                    ins=[fc_part], outs=[fc_full])
                fcT = pp.tile([OUT_CH, N_GRAPHS], f32)
                nc.sync.dma_start(fcT[:], fc_full[:])
                for b in range(N_GRAPHS // P):
                    tb_ps = psX.tile([P, OUT_CH], f32, space="PSUM", tag="mm", name=f"tbp{b}")
                    nc.tensor.matmul(tb_ps[:], lhsT=fcT[:, b * P:(b + 1) * P],
                                     rhs=ident[:OUT_CH, :OUT_CH],
                                     is_transpose=True, start=True, stop=True)
                    sc = wp.tile([P, OUT_CH], f32, tag="sc")
                    nc.vector.tensor_scalar(sc[:], tb_ps[:], invc[:, b:b + 1],
                                            None, OP.mult)
                    ad = wp.tile([P, OUT_CH], f32, tag="ad")
                    nc.vector.tensor_tensor(ad[:], sc[:], bfc_sb[:], op=OP.add)
                    sg = wp.tile([P, OUT_CH], f32, tag="sg")
                    nc.scalar.activation(sg[:], ad[:], AF.Sigmoid)
                    nc.sync.dma_start(out_d[b * P:(b + 1) * P, :], sg[:])

            stage_limit = int(os.environ.get("K_STAGE", "0"))

            def dbg_out():
                for b in range(N_GRAPHS // P):
                    t = wp.tile([P, OUT_CH], f32, tag="dbg", name=f"dbg{b}")
                    nc.vector.tensor_copy(t[:], b2_sb[:, 0:OUT_CH])
                    nc.sync.dma_start(out_d[b * P:(b + 1) * P, :], t[:])

            stages = [
                lambda: matmul_stage(0, 0, FQ0),
                lambda: (allgather(0, 0), build_sp()),
                lambda: matmul_stage(0, FQ0, NF),
                lambda: allgather(0, 1),
                lambda: agg_pass(0, 0),
                lambda: agg_pass(0, 1),
                lambda: matmul_stage(1, 0, FQ0),
                lambda: allgather(1, 0),
                lambda: matmul_stage(1, FQ0, NF),
                lambda: allgather(1, 1),
                lambda: agg_pass(1, 0),
                lambda: agg_pass(1, 1),
                pool_fc,
            ]
            nstage = stage_limit if stage_limit > 0 else len(stages)
            for s in stages[:nstage]:
                s()
            if nstage < len(stages):
                dbg_out()

    nc.compile()
    return nc


def _in_maps_for(prep, x, W1, b1, W2, b2, Wfc, bfc):
    bf = ml_dtypes.bfloat16
    xT = np.zeros((N_CORES, P, SHP), dtype=bf)
    for c in range(N_CORES):
        xT[c, :, :SH] = x[c * SH:(c + 1) * SH].T
    b1c = b1.reshape(P, 1)
    b2r = np.broadcast_to(b2.reshape(1, HID), (P, HID)).copy()
    bfcr = np.broadcast_to(bfc.reshape(1, OUT_CH), (P, OUT_CH)).copy()
    in_maps = []
    for c in range(N_CORES):
        in_maps.append({
            "xT_sh": xT[c], "W1": W1, "W2": W2, "Wfc": Wfc,
            "b1c": b1c, "b2r": b2r, "bfcr": bfcr,
            "idx_in": np.ascontiguousarray(prep["idx_all"][c]),
            "drel_in": np.ascontiguousarray(prep["drel_tab"][c]),
            "gid_in": np.ascontiguousarray(prep["gid_tab"][c]),
            "dinv_in": np.ascontiguousarray(prep["dinv_sh"][c]),
            "invc_in": np.ascontiguousarray(prep["invc_t"]),
        })
    return in_maps


def kernel(x, edge_index, batch, W1, b1, W2, b2, Wfc, bfc):
    from concourse.bass_utils import run_bass_kernel_spmd

    x = np.asarray(x, dtype=np.float32)
    W1 = np.asarray(W1, dtype=np.float32)
    W2 = np.asarray(W2, dtype=np.float32)
    Wfc = np.asarray(Wfc, dtype=np.float32)
    b1 = np.asarray(b1, dtype=np.float32)
    b2 = np.asarray(b2, dtype=np.float32)
    bfc = np.asarray(bfc, dtype=np.float32)

    key = (int(np.asarray(edge_index).sum()) & 0xFFFFFFFF,)
    if key not in _CACHE:
        prep = _host_prep(edge_index, batch)
        prog = _build_program(prep)
        _CACHE[key] = (prep, prog)
    prep, prog = _CACHE[key]

    in_maps = _in_maps_for(prep, x, W1, b1, W2, b2, Wfc, bfc)
    global LAST_RESULT
    res = run_bass_kernel_spmd(prog, in_maps, core_ids=list(range(N_CORES)))
    LAST_RESULT = res
    return np.asarray(res.results[0]["out"], dtype=np.float32)


# revision 31
# speedup vs baseline: 1.1924x; 1.0125x over previous
"""GCN (2x GCNConv + global_mean_pool + FC + sigmoid) on 8 TRN2 NeuronCores.

Sharding: nodes (and incident edges, by dst) are partitioned across 8 cores.
Each core computes the feature transform + message aggregation for its 6250
dst nodes; hs (dinv-scaled transformed features, fp8-e4m3) is AllGathered
between layers; the [16, 512] FC partials are AllReduced at the end.

fp8 message path: hs rows are stored pair-packed ([n/2, 256] fp8 — two nodes
per 256-byte row) because dma_gather requires 256B-aligned elements. Each
128-slot aggregation tile holds only even-src or only odd-src edges; the
one-hot S matmul for an even tile reads bytes 0:128 of the gathered pair
rows, an odd tile reads bytes 128:256, so every matmul keeps K=128 (partial-
partition matmuls are flaky on HW). Pair row ids all fit int16
(25088 < 32768) so there is no lo/hi index split. Gather calls are capped at
1024 indices — the SWDGE descriptor ring holds 1024 and a larger single call
wedges the device.

The one-hot S (edge-slot -> dst) and Sp (node -> graph) matrices are built
on device by DVE iota-compare from compact bf16 id tables instead of being
shipped as multi-MB inputs and re-streamed from HBM during aggregation.

Host does integer-only graph preprocessing; all floating-point math runs on
device. fp8 quantization noise averages out in aggregation + pooling
(measured ~1e-4 end-to-end rel err vs the fp32 reference; gate is 2e-2).
"""
import numpy as np
import ml_dtypes

N_NODES = 50000
N_EDGES = 600000
HID = 128
OUT_CH = 16
N_GRAPHS = 512
N_CORES = 8
P = 128
SH = N_NODES // N_CORES          # 6250 nodes per shard
NF = (SH + P - 1) // P           # 49 frames of 128 nodes
SHP = NF * P                     # 6272 padded shard rows
NPAIR = SHP // 2                 # 3136 pair rows per shard
NFULL = N_CORES * SHP            # 50176 padded gather-table rows
CF = 6                           # frames per aggregation chunk (PSUM banks)
GT = 8                           # gather tiles per call (1024-desc ring cap)
FQ0 = 25                         # frames in src-chunk 0 (AllGather split)
PQ_SPLIT = FQ0 * (P // 2)        # 1600 pair rows in chunk 0
RQ = (PQ_SPLIT, NPAIR - PQ_SPLIT)  # pair rows per chunk (1600, 1536)
EMPTY_DREL = 200.0               # slot-empty sentinel (never equals 0..127)
EMPTY_GID = 600                # node-pad sentinel (never equals 0..511)

_CACHE = {}
LAST_RESULT = None  # test.py reads exec_time_ns / trace path from here


def _host_prep(edge_index, batch):
    src = np.asarray(edge_index[0], dtype=np.int64)
    dst = np.asarray(edge_index[1], dtype=np.int64)
    batch = np.asarray(batch, dtype=np.int64)
    bf = ml_dtypes.bfloat16

    deg = np.bincount(dst, minlength=N_NODES) + 1  # + self loop

    # padded gather-table row id for each node
    prow = (np.arange(N_NODES) // SH) * SHP + (np.arange(N_NODES) % SH)

    # edges incl. self loops, keyed by (dst core, dst frame, src parity)
    all_src = np.concatenate([src, np.arange(N_NODES)])
    all_dst = np.concatenate([dst, np.arange(N_NODES)])
    core_of = all_dst // SH
    frame_of = (all_dst % SH) // P
    dstrel = (all_dst % SH) % P
    srow = prow[all_src]
    pair = srow // 2
    par = srow % 2

    lp = (srow % SHP) // 2              # local pair row within shard
    qof = (lp >= PQ_SPLIT).astype(np.int64)
    rel = (srow // SHP) * 0  # placeholder
    rel = (srow // SHP) * RQ[0]
    rel = np.where(qof == 0, (srow // SHP) * RQ[0] + lp,
                   (srow // SHP) * RQ[1] + (lp - PQ_SPLIT))

    key = ((core_of * NF + frame_of) * 2 + qof) * 2 + par
    o = np.argsort(key, kind="stable")
    ksort = key[o]
    srt_rel = rel[o]
    srt_drel = dstrel[o]
    cuts = np.searchsorted(ksort, np.arange(N_CORES * NF * 4 + 1))
    cnts = (cuts[1:] - cuts[:-1]).reshape(N_CORES, NF, 2, 2)

    # tiles per (frame, q, parity): uniform across cores (SPMD-identical
    # program); each 128-slot tile holds edges of one (src-chunk, parity)
    t_fqp = np.maximum((cnts.max(axis=0) + P - 1) // P, 1)  # [NF, 2, 2]

    chunks = []
    f = 0
    while f < NF:
        chunks.append(list(range(f, min(f + CF, NF))))
        f += CF

    ntiles_total = int(t_fqp.sum())
    nslots = ntiles_total * P

    idx_all = np.zeros((N_CORES, P, nslots // 16), dtype=np.int16)
    drel_tab = np.full((N_CORES, P, ntiles_total), EMPTY_DREL, dtype=bf)
    tile_base = 0
    chunk_meta = []  # per chunk: (tile_base, frames, frame->{q:(ev,od)})
    for fr in chunks:
        spans = {fi: {} for fi in fr}
        tb = tile_base
        for q in (0, 1):         # q-major: all q0 tiles, then all q1 tiles
            for fi in fr:
                ev = list(range(tb, tb + int(t_fqp[fi, q, 0])))
                tb += int(t_fqp[fi, q, 0])
                od = list(range(tb, tb + int(t_fqp[fi, q, 1])))
                tb += int(t_fqp[fi, q, 1])
                spans[fi][q] = (ev, od)
        chunk_meta.append((tile_base, fr, spans))
        tile_base = tb
    assert tile_base == ntiles_total

    frame_tiles = {}
    for (_, fr, spans) in chunk_meta:
        for fi in fr:
            frame_tiles[fi] = spans[fi]

    for c in range(N_CORES):
        for fi in range(NF):
            for q in (0, 1):
                for half in (0, 1):  # 0 = even srow (bytes 0:128), 1 = odd
                    tiles = frame_tiles[fi][q][half]
                    k = ((c * NF + fi) * 2 + q) * 2 + half
                    e0, e1 = cuts[k], cuts[k + 1]
                    rows = srt_rel[e0:e1]
                    drel = srt_drel[e0:e1]
                    n = e1 - e0
                    assert n <= len(tiles) * P
                    for j in range(n):
                        t = tiles[j // P]
                        e = j % P
                        drel_tab[c, e, t] = drel[j]
                        slot = t * P + e
                        idx_all[c, slot % 16, slot // 16] = rows[j]
    # replicate idx rows 0..15 to the other 7 groups of 16 partitions
    for g in range(1, 8):
        idx_all[:, 16 * g: 16 * (g + 1), :] = idx_all[:, 0:16, :]

    # dinv per shard, [128, NF] (node f*128+s -> [s, f]), pad -> 1.0
    dinv_sh = np.ones((N_CORES, P, NF), dtype=np.float32)
    for c in range(N_CORES):
        d = deg[c * SH:(c + 1) * SH].astype(np.float32)
        dp = np.concatenate([d, np.ones(SHP - SH, np.float32)])
        dinv_sh[c] = (1.0 / np.sqrt(dp)).reshape(NF, P).T

    # graph id per node, [128, NF] int16 (pad -> sentinel; bf16 cannot
    # represent odd ids >= 256)
    gid_tab = np.full((N_CORES, P, NF), EMPTY_GID, dtype=np.int16)
    for c in range(N_CORES):
        b = batch[c * SH:(c + 1) * SH]
        bp = np.concatenate([b, np.full(SHP - SH, EMPTY_GID, np.int64)])
        gid_tab[c] = bp.reshape(NF, P).T.astype(np.int16)

    cnt = np.maximum(np.bincount(batch, minlength=N_GRAPHS), 1)
    invc_t = (1.0 / cnt.astype(np.float32)).reshape(N_GRAPHS // P, P).T

    return dict(idx_all=idx_all, drel_tab=drel_tab, gid_tab=gid_tab,
                dinv_sh=dinv_sh, invc_t=invc_t, frame_tiles=frame_tiles,
                ntiles_total=ntiles_total, chunk_meta=chunk_meta, t_fqp=t_fqp)


def _build_program(prep):
    import os
    import concourse.tile as tile
    from concourse import bacc, mybir
    from concourse.masks import make_identity

    ntiles = prep["ntiles_total"]
    chunk_meta = prep["chunk_meta"]
    ctmax = max(
        sum(len(s[fi][q][0]) + len(s[fi][q][1]) for fi in fr)
        for (_, fr, s) in chunk_meta for q in (0, 1))

    nc = bacc.Bacc("TRN2", target_bir_lowering=False, debug=False,
                   num_devices=N_CORES)
    f32, bf16 = mybir.dt.float32, mybir.dt.bfloat16
    f8 = mybir.dt.float8e4
    i16 = mybir.dt.int16
    AF = mybir.ActivationFunctionType
    OP = mybir.AluOpType

    # ---- IO ----
    xT_in = nc.dram_tensor("xT_sh", [P, SHP], bf16, kind="ExternalInput").ap()
    W1 = nc.dram_tensor("W1", [HID, HID], f32, kind="ExternalInput").ap()
    W2 = nc.dram_tensor("W2", [HID, HID], f32, kind="ExternalInput").ap()
    Wfc = nc.dram_tensor("Wfc", [HID, OUT_CH], f32, kind="ExternalInput").ap()
    b1c = nc.dram_tensor("b1c", [P, 1], f32, kind="ExternalInput").ap()
    b2r = nc.dram_tensor("b2r", [P, HID], f32, kind="ExternalInput").ap()
    bfcr = nc.dram_tensor("bfcr", [P, OUT_CH], f32, kind="ExternalInput").ap()
    idx_in = nc.dram_tensor("idx_in", [P, ntiles * P // 16], i16, kind="ExternalInput").ap()
    drel_in = nc.dram_tensor("drel_in", [P, ntiles], bf16, kind="ExternalInput").ap()
    gid_in = nc.dram_tensor("gid_in", [P, NF], i16, kind="ExternalInput").ap()
    dinv_in = nc.dram_tensor("dinv_in", [P, NF], f32, kind="ExternalInput").ap()
    invc_in = nc.dram_tensor("invc_in", [P, N_GRAPHS // P], f32, kind="ExternalInput").ap()
    out_d = nc.dram_tensor("out", [N_GRAPHS, OUT_CH], f32, kind="ExternalOutput").ap()

    # internal DRAM: pair-packed hs (two fp8 node rows per 256B row);
    # hs_q[l][q] holds the AllGathered src-chunk q (separate tensors so the
    # chunk-0 gathers never wait on the chunk-1 AllGather)
    hs_sh = [nc.dram_tensor(f"hs_sh{l}", [NPAIR, 2 * HID], f8, kind="Internal").ap()
             for l in range(2)]
    hs_q = [[nc.dram_tensor(f"hs_q{l}_{q}", [N_CORES * RQ[q], 2 * HID], f8,
                            kind="Internal").ap() for q in (0, 1)]
            for l in range(2)]
    fc_part = nc.dram_tensor("fc_part", [OUT_CH, N_GRAPHS], f32, kind="Internal").ap()
    fc_full = nc.dram_tensor("fc_full", [OUT_CH, N_GRAPHS], f32, kind="Internal").ap()

    with tile.TileContext(nc, num_cores=N_CORES) as tc:
        with tc.tile_pool(name="const", bufs=1) as cp, \
             tc.tile_pool(name="persist", bufs=1) as pp, \
             tc.tile_pool(name="work", bufs=3) as wp, \
             tc.tile_pool(name="msgs", bufs=2) as mp, \
             tc.tile_pool(name="psAcc", bufs=6, space="PSUM") as psAcc, \
             tc.tile_pool(name="psX", bufs=2, space="PSUM") as psX, \
             tc.tile_pool(name="dram", bufs=2, space="DRAM") as dp:

            # ---- constants ----
            ident = cp.tile([P, P], f32)
            make_identity(nc, ident[:])
            W1b = cp.tile([P, HID], bf16)
            W2b = cp.tile([P, HID], bf16)
            Wfb = cp.tile([P, OUT_CH], bf16)
            for Wd, Wb in ((W1, W1b), (W2, W2b), (Wfc, Wfb)):
                wf = wp.tile([P, Wd.shape[1]], f32, tag="wtmp")
                nc.sync.dma_start(wf[:], Wd[:])
                nc.vector.tensor_copy(Wb[:], wf[:])
            b1_sb = cp.tile([P, 1], f32)
            nc.sync.dma_start(b1_sb[:], b1c[:])
            b2_sb = cp.tile([P, HID], f32)
            nc.sync.dma_start(b2_sb[:], b2r[:])
            bfc_sb = cp.tile([P, OUT_CH], f32)
            nc.sync.dma_start(bfc_sb[:], bfcr[:])
            # x shard, channel-major (pre-transposed on host)
            xt_sb = pp.tile([P, SHP], bf16)
            nc.sync.dma_start(xt_sb[:], xT_in[:])
            dinv = cp.tile([P, NF], f32)
            nc.sync.dma_start(dinv[:], dinv_in[:])
            invc = cp.tile([P, N_GRAPHS // P], f32)
            nc.sync.dma_start(invc[:], invc_in[:])
            # id tables + the whole gather-index table (resident in SBUF)
            drel_sb = cp.tile([P, ntiles], bf16)
            nc.sync.dma_start(drel_sb[:], drel_in[:])
            gid_sb = cp.tile([P, NF], i16)
            nc.sync.dma_start(gid_sb[:], gid_in[:])
            idx_sb = pp.tile([P, ntiles * P // 16], i16)
            nc.sync.dma_start(idx_sb[:], idx_in[:])
            # iota ramps for the one-hot builds
            iota_rep = cp.tile([P, ctmax, P], bf16)  # 0..127 per tile
            nc.gpsimd.iota(iota_rep[:], pattern=[[0, ctmax], [1, P]], base=0,
                           channel_multiplier=0,
                           allow_small_or_imprecise_dtypes=True)
            iota_g = cp.tile([P, N_GRAPHS], i16)     # 0..511
            nc.gpsimd.iota(iota_g[:], pattern=[[1, N_GRAPHS]], base=0,
                           channel_multiplier=0,
                           allow_small_or_imprecise_dtypes=True)

            sp_all = pp.tile([P, NF, N_GRAPHS], f8)  # pooling one-hots

            def build_sp():
                # emitted after the first AllGather so the DVE work lands in
                # the collective's idle window, not ahead of the mm scales
                for f in range(NF):
                    nc.vector.tensor_tensor(
                        sp_all[:, f, :],
                        gid_sb[:, f:f + 1].broadcast_to([P, N_GRAPHS]),
                        iota_g[:], op=OP.is_equal)

            dinv_b = pp.tile([P, NF, HID], bf16)  # dinv broadcast per frame
            nc.vector.tensor_copy(
                dinv_b[:], dinv[:].unsqueeze(2).broadcast_to([P, NF, HID]))

            out1T = pp.tile([P, SHP], bf16)   # layer-1 output, ch-major
            out2 = pp.tile([P, NF, HID], bf16)  # layer-2 output, node-major

            # ---- layer matmul stages ----
            # 4 frames share one PSUM bank + one fused DVE scale + one DMA
            # (per-frame sync chains dominated the stage otherwise)
            def matmul_stage(layer, f0, f1):
                for b0 in range(f0, f1, 4):
                    b1 = min(b0 + 4, f1)
                    F = b1 - b0
                    u_ps = psX.tile([P, F, HID], f32, space="PSUM", tag="mm",
                                    name=f"u{layer}_{b0}")
                    for b in range(b0, b1):
                        lhs_ap = (xt_sb if layer == 0 else out1T)[:, b * P:(b + 1) * P]
                        Wb = W1b if layer == 0 else W2b
                        nc.tensor.matmul(u_ps[:, b - b0, :], lhsT=lhs_ap,
                                         rhs=Wb[:], start=True, stop=True,
                                         skip_group_check=True)
                    hsb = wp.tile([P, F, HID], f8, tag="hsb")
                    nc.vector.tensor_tensor(hsb[:], u_ps[:],
                                            dinv_b[:, b0:b1, :], op=OP.mult)
                    # pair rows [b0*64, b1*64); dram order is node-major,
                    # sbuf is partition-major -> reorder via the dst AP
                    dst = hs_sh[layer][b0 * (P // 2):b1 * (P // 2), :]
                    dst = dst.rearrange("(f n2) (two h) -> (n2 two) f h",
                                        f=F, two=2)
                    nc.sync.dma_start(dst, hsb[:])

            def allgather(layer, q):
                p0 = 0 if q == 0 else PQ_SPLIT
                nc.gpsimd.collective_compute(
                    "AllGather", OP.bypass,
                    replica_groups=[list(range(N_CORES))],
                    ins=[hs_sh[layer][p0:p0 + RQ[q], :]], outs=[hs_q[layer][q]],
                )

            # ---- aggregation stage ----
            # f32 accumulators for the two-pass aggregation (q0 sums
            # buffered here while the q1 AllGather is still in flight)
            acc_all = pp.tile([P, NF, HID], f32)

            def agg_pass(layer, q):
                k_agg = os.environ.get("K_AGG", "")
                src = hs_q[layer][q]
                for (tb, fr, spans) in chunk_meta:
                    qt = [t for fi in fr for t in spans[fi][q][0] + spans[fi][q][1]]
                    q0, q1t = min(qt), max(qt) + 1  # q-tiles are contiguous
                    ct = q1t - q0
                    msg = mp.tile([P, ct, 2 * HID], f8, tag="msg")
                    # one-hot S for this chunk's q-tiles, built on DVE
                    s_sb = mp.tile([P, ct, P], f8, tag="S")
                    nc.vector.tensor_tensor(
                        s_sb[:],
                        drel_sb[:, q0:q1t].unsqueeze(2).broadcast_to([P, ct, P]),
                        iota_rep[:, 0:ct, :],
                        op=OP.is_equal)
                    if k_agg == "dma":
                        continue
                    # gather in <=1024-idx calls (SWDGE ring capacity)
                    for g0 in range(0, ct, GT):
                        g1 = min(g0 + GT, ct)
                        nc.gpsimd.dma_gather(
                            out_ap=msg[:, g0:g1, :], in_ap=src[:],
                            idxs_ap=idx_sb[:, (q0 + g0) * P // 16:(q0 + g1) * P // 16],
                            num_idxs=(g1 - g0) * P, num_idxs_reg=(g1 - g0) * P,
                            elem_size=2 * HID)
                    if k_agg == "gather":
                        continue
                    accs = {}
                    for fi in fr:
                        accs[fi] = psAcc.tile([P, HID], f32, space="PSUM", tag="acc", name=f"acc{layer}_{q}_{fi}")
                    # absorber: single dummy matmul observes S + msg + acc sems
                    nc.tensor.matmul(accs[fr[0]][0:2, 0:2], lhsT=s_sb[:, 0, 0:2],
                                     rhs=msg[:, 0, 0:2], start=True, stop=True,
                                     skip_group_check=True)
                    # per frame: one K=128 matmul per tile; even-src tiles
                    # read bytes 0:128 of the gathered pair row, odd 128:256
                    for fi in fr:
                        ev, od = spans[fi][q]
                        nmm = len(ev) + len(od)
                        j = 0
                        for t, c0 in [(t, 0) for t in ev] + [(t, HID) for t in od]:
                            tl = t - q0
                            nc.tensor.matmul(
                                accs[fi][:],
                                lhsT=s_sb[:, tl, :],
                                rhs=msg[:, tl, c0:c0 + HID],
                                start=(j == 0),
                                stop=(q == 0 and j == nmm - 1),
                                skip_group_check=True)
                            j += 1
                        if q == 1:
                            # fold the buffered q0 sums into the PSUM group on
                            # the idle PE instead of a DVE add per frame
                            nc.tensor.matmul(
                                accs[fi][:], lhsT=ident[:],
                                rhs=acc_all[:, fi, :],
                                start=False, stop=True,
                                skip_group_check=True)
                    if k_agg == "mm":
                        continue
                    for fi in fr:
                        if q == 0:
                            nc.vector.tensor_copy(acc_all[:, fi, :], accs[fi][:])
                            continue
                        ags = wp.tile([P, HID], f32, tag="ags")
                        nc.vector.tensor_scalar(ags[:], accs[fi][:],
                                                dinv[:, fi:fi + 1], None, OP.mult)
                        if layer == 0:
                            agT = psX.tile([P, P], f32, space="PSUM", tag="mm", name=f"agT{fi}")
                            nc.tensor.transpose(agT[:], ags[:], ident[:])
                            nc.scalar.activation(
                                out1T[:, fi * P:(fi + 1) * P], agT[:],
                                AF.Relu, bias=b1_sb[:, 0:1])
                        else:
                            ab = wp.tile([P, HID], f32, tag="ab")
                            nc.vector.tensor_tensor(ab[:], ags[:], b2_sb[:],
                                                    op=OP.add)
                            nc.scalar.activation(out2[:, fi, :], ab[:], AF.Relu)

            # ---- pooling + FC (FC partials AllReduced, [16, 512] f32) ----
            def pool_fc():
                pl_ps = psX.tile([P, N_GRAPHS], f32, space="PSUM", tag="mm", name="pl_ps")
                nc.tensor.matmul(pl_ps[0:1, 0:1], lhsT=out2[:, 0, 0:1],
                                 rhs=out2[:, 0, 0:1], start=True, stop=True,
                                 skip_group_check=True)
                for f in range(NF):
                    nc.tensor.matmul(pl_ps[:], lhsT=out2[:, f, :],
                                     rhs=sp_all[:, f, :],
                                     start=(f == 0), stop=(f == NF - 1),
                                     skip_group_check=True)
                pf = wp.tile([P, N_GRAPHS], bf16, tag="plsb")
                nc.vector.tensor_copy(pf[:], pl_ps[:])
                fc_ps = psX.tile([OUT_CH, N_GRAPHS], f32, space="PSUM", tag="mm", name="fc_ps")
                nc.tensor.matmul(fc_ps[:], lhsT=Wfb[:], rhs=pf[:],
                                 start=True, stop=True)
                fcp = wp.tile([OUT_CH, N_GRAPHS], f32, tag="fcp")
                nc.vector.tensor_copy(fcp[:], fc_ps[:])
                nc.sync.dma_start(fc_part[:], fcp[:])
                nc.gpsimd.collective_compute(
                    "AllReduce", OP.add, replica_groups=[list(range(N_CORES))],
                    ins=[fc_part], outs=[fc_full])
                fcT = pp.tile([OUT_CH, N_GRAPHS], f32)
                nc.sync.dma_start(fcT[:], fc_full[:])
                for b in range(N_GRAPHS // P):
                    tb_ps = psX.tile([P, OUT_CH], f32, space="PSUM", tag="mm", name=f"tbp{b}")
                    nc.tensor.matmul(tb_ps[:], lhsT=fcT[:, b * P:(b + 1) * P],
                                     rhs=ident[:OUT_CH, :OUT_CH],
                                     is_transpose=True, start=True, stop=True)
                    sc = wp.tile([P, OUT_CH], f32, tag="sc")
                    nc.vector.tensor_scalar(sc[:], tb_ps[:], invc[:, b:b + 1],
                                            None, OP.mult)
                    ad = wp.tile([P, OUT_CH], f32, tag="ad")
                    nc.vector.tensor_tensor(ad[:], sc[:], bfc_sb[:], op=OP.add)
                    sg = wp.tile([P, OUT_CH], f32, tag="sg")
                    nc.scalar.activation(sg[:], ad[:], AF.Sigmoid)
                    nc.sync.dma_start(out_d[b * P:(b + 1) * P, :], sg[:])

            stage_limit = int(os.environ.get("K_STAGE", "0"))

            def dbg_out():
                for b in range(N_GRAPHS // P):
                    t = wp.tile([P, OUT_CH], f32, tag="dbg", name=f"dbg{b}")
                    nc.vector.tensor_copy(t[:], b2_sb[:, 0:OUT_CH])
                    nc.sync.dma_start(out_d[b * P:(b + 1) * P, :], t[:])

            stages = [
                lambda: matmul_stage(0, 0, FQ0),
                lambda: (allgather(0, 0), build_sp()),
                lambda: matmul_stage(0, FQ0, NF),
                lambda: allgather(0, 1),
                lambda: agg_pass(0, 0),
                lambda: agg_pass(0, 1),
                lambda: matmul_stage(1, 0, FQ0),
                lambda: allgather(1, 0),
                lambda: matmul_stage(1, FQ0, NF),
                lambda: allgather(1, 1),
                lambda: agg_pass(1, 0),
                lambda: agg_pass(1, 1),
                pool_fc,
            ]
            nstage = stage_limit if stage_limit > 0 else len(stages)
            for s in stages[:nstage]:
                s()
            if nstage < len(stages):
                dbg_out()

    nc.compile()
    return nc


def _in_maps_for(prep, x, W1, b1, W2, b2, Wfc, bfc):
    bf = ml_dtypes.bfloat16
    xT = np.zeros((N_CORES, P, SHP), dtype=bf)
    for c in range(N_CORES):
        xT[c, :, :SH] = x[c * SH:(c + 1) * SH].T
    b1c = b1.reshape(P, 1)
    b2r = np.broadcast_to(b2.reshape(1, HID), (P, HID)).copy()
    bfcr = np.broadcast_to(bfc.reshape(1, OUT_CH), (P, OUT_CH)).copy()
    in_maps = []
    for c in range(N_CORES):
        in_maps.append({
            "xT_sh": xT[c], "W1": W1, "W2": W2, "Wfc": Wfc,
            "b1c": b1c, "b2r": b2r, "bfcr": bfcr,
            "idx_in": np.ascontiguousarray(prep["idx_all"][c]),
            "drel_in": np.ascontiguousarray(prep["drel_tab"][c]),
            "gid_in": np.ascontiguousarray(prep["gid_tab"][c]),
            "dinv_in": np.ascontiguousarray(prep["dinv_sh"][c]),
            "invc_in": np.ascontiguousarray(prep["invc_t"]),
        })
    return in_maps


def kernel(x, edge_index, batch, W1, b1, W2, b2, Wfc, bfc):
    from concourse.bass_utils import run_bass_kernel_spmd

    x = np.asarray(x, dtype=np.float32)
    W1 = np.asarray(W1, dtype=np.float32)
    W2 = np.asarray(W2, dtype=np.float32)
    Wfc = np.asarray(Wfc, dtype=np.float32)
    b1 = np.asarray(b1, dtype=np.float32)
    b2 = np.asarray(b2, dtype=np.float32)
    bfc = np.asarray(bfc, dtype=np.float32)

    key = (int(np.asarray(edge_index).sum()) & 0xFFFFFFFF,)
    if key not in _CACHE:
        prep = _host_prep(edge_index, batch)
        prog = _build_program(prep)
        _CACHE[key] = (prep, prog)
    prep, prog = _CACHE[key]

    in_maps = _in_maps_for(prep, x, W1, b1, W2, b2, Wfc, bfc)
    global LAST_RESULT
    res = run_bass_kernel_spmd(prog, in_maps, core_ids=list(range(N_CORES)))
    LAST_RESULT = res
    return np.asarray(res.results[0]["out"], dtype=np.float32)
